# revision 1
# baseline (speedup 1.0000x reference)
"""Bidirectional 2-layer LSTM (B=32, T=256, IN=H=512) on 8 Trainium2 cores.

Strategy:
  - Directions are anti-aligned in time, so fwd/bwd run on separate cores
    (even cores = fwd, odd = bwd) with bwd fed time-reversed inputs.
  - Two SPMD launches: launch 1 computes layer-0 input gates + recurrence,
    launch 2 (after a host roundtrip that exchanges/reverses h0 between
    directions) computes layer-1 the same way. Both launches share one NEFF
    builder; the only structural difference is the contraction depth of the
    input-gate GEMM (512 vs 1024).
  - Per-core layout keeps the hidden state as [Hw=128 part, (hb,b)=128 free]
    fp16 so the recurrent matmul (w_hh stationary, h moving) needs no
    transposes anywhere in the 256-step serial chain.
  - Gate tile order is [f, i, o, g] so one Sigmoid covers f,i,o and one Tanh
    covers g. Cell state stays fp32; matmul operands fp16 with fp32 PSUM.
"""

import os
import sys

for _p in ("/opt/trn_rl_repo", "/root/.axon_site/_ro/trn_rl_repo"):
    if os.path.isdir(_p) and _p not in sys.path:
        sys.path.insert(0, _p)

import numpy as np

import concourse.bass as bass
import concourse.bacc as bacc
import concourse.tile as tile
import concourse.mybir as mybir
import concourse.bass_utils as bass_utils

NCORES = 8
B, T, IN, H = 32, 256, 512, 512
T = int(os.environ.get("LSTM_T", T))  # test override; harness uses 256
NSWEEP = T // 16     # t-sweeps in the input-gate GEMM
TSW = 16             # timesteps per sweep
F16 = mybir.dt.float16
F32 = mybir.dt.float32

# gate tile order [f, i, g, o]; OG maps tile-group -> original gate index
# in the reference's [i, f, g, o] column order.
OG = (1, 0, 2, 3)
_PERM = np.concatenate(
    [512 * OG[m // 4] + 128 * (m % 4) + np.arange(128) for m in range(16)]
)

_NC_CACHE = {}


def _build_launch(nkc):
    """One direction-layer: xg GEMM (nkc K-chunks of 128) + 256-step LSTM
    recurrence. Inputs (per core):
      rhs   [NSWEEP, nkc, 128, 512] f16 -- moving operand tiles, (t16,b32) cols
      wih   [nkc, 16, 128, 128] f16     -- input-weight tiles (col-permuted)
      whh   [4, 16, 128, 128] f16       -- recurrent-weight tiles
      biasq [4, 128, 2048] f32          -- bias pre-broadcast per m-quarter
    Output:
      hout  [T, 128, 128] f16           -- h_t in [Hw, (hb,b)] layout
    """
    nc = bacc.Bacc("TRN2", target_bir_lowering=False, debug=False,
                   enable_asserts=True, num_devices=NCORES)
    rhs_d = nc.dram_tensor("rhs", [NSWEEP, nkc, 128, 512], F16,
                           kind="ExternalInput")
    wih_d = nc.dram_tensor("wih", [nkc, 16, 128, 128], F16,
                           kind="ExternalInput")
    whh_d = nc.dram_tensor("whh", [4, 16, 128, 128], F16,
                           kind="ExternalInput")
    biasq_d = nc.dram_tensor("biasq", [4, 128, 2048], F32,
                             kind="ExternalInput")
    hout_d = nc.dram_tensor("hout", [T, 128, 128], F16,
                            kind="ExternalOutput")

    with tile.TileContext(nc) as tc:
        with (
            tc.tile_pool(name="wpool", bufs=1) as wpool,
            tc.tile_pool(name="dram", bufs=1, space="DRAM") as dram,
        ):
            # resident weights / bias
            wih_sb = wpool.tile([128, nkc * 16 * 128], F16)
            whh_sb = wpool.tile([128, 4 * 16 * 128], F16)
            biasq_sb = wpool.tile([128, 4 * 2048], F32)
            nc.sync.dma_start(
                wih_sb[:].rearrange("k (c m j) -> k c m j",
                                      c=nkc, m=16, j=128),
                wih_d.ap().rearrange("c m k j -> k c m j"))
            nc.sync.dma_start(
                whh_sb[:].rearrange("k (c m j) -> k c m j",
                                      c=4, m=16, j=128),
                whh_d.ap().rearrange("c m k j -> k c m j"))
            nc.sync.dma_start(
                biasq_sb[:].rearrange("k (q j) -> k q j", q=4, j=2048),
                biasq_d.ap().rearrange("q k j -> k q j"))

            def wih_t(c, m):
                o = (c * 16 + m) * 128
                return wih_sb[:, o:o + 128]

            def whh_t(c, m):
                o = (c * 16 + m) * 128
                return whh_sb[:, o:o + 128]

            xg_store = dram.tile([T, 128, 512], F16)

            # ---- xg GEMM interleaved with the recurrence ----
            # The rec step leaves the PE idle during its elementwise tail;
            # emitting one m-tile of a future sweep's GEMM per rec step fills
            # those gaps, making the input-gate GEMM (nearly) free. Two
            # sweeps are emitted up front so xg stays 2 sweeps ahead.
            with (
                tc.tile_pool(name="rt", bufs=2 * nkc) as rtp,
                tc.tile_pool(name="apsum", bufs=2, space="PSUM") as apsum,
                tc.tile_pool(name="xgsb", bufs=2) as xgp,
                tc.tile_pool(name="state", bufs=1) as statep,
                tc.tile_pool(name="xgt", bufs=4) as xgtp,
                tc.tile_pool(name="rpsum", bufs=2, space="PSUM") as rpsum,
                tc.tile_pool(name="gates", bufs=2) as gatesp,
                tc.tile_pool(name="cell", bufs=2) as cellp,
                tc.tile_pool(name="hbuf", bufs=2) as hbufp,
                tc.tile_pool(name="tmp", bufs=4) as tmpp,
            ):
                sweep_ctx = {}

                def start_sweep(s):
                    rts = []
                    for c in range(nkc):
                        rt = rtp.tile([128, 512], F16, name="rt")
                        nc.sync.dma_start(rt[:], rhs_d[s, c])
                        rts.append(rt)
                    xg_sb = xgp.tile([128, TSW * 512], F16, name="xg_sb")
                    sweep_ctx[s] = (rts, xg_sb)

                def emit_unit(s, m):
                    rts, xg_sb = sweep_ctx[s]
                    ps = apsum.tile([128, 512], F32, name="aps", tag="aps")
                    for c in range(nkc):
                        nc.tensor.matmul(ps[:], wih_t(c, m), rts[c][:],
                                         start=(c == 0), stop=(c == nkc - 1))
                    src = ps[:].rearrange("k (t b) -> k t b", t=TSW, b=32)
                    dst = xg_sb[:].rearrange("k (t m b) -> k m t b",
                                             t=TSW, m=16, b=32)[:, m]
                    q, mm = m // 4, m % 4
                    bias = biasq_sb[:, 2048 * q:2048 * (q + 1)].rearrange(
                        "k (m t b) -> k m t b", m=4, t=TSW, b=32)[:, mm]
                    nc.vector.tensor_add(dst, src, bias)

                def flush_sweep(s):
                    _, xg_sb = sweep_ctx.pop(s)
                    nc.sync.dma_start(
                        xg_store[TSW * s:TSW * (s + 1)].rearrange(
                            "t k c -> k t c"),
                        xg_sb[:].rearrange("k (t c) -> k t c",
                                           t=TSW, c=512))

                upfront = min(2, NSWEEP)
                for s in range(upfront):
                    start_sweep(s)
                    for m in range(16):
                        emit_unit(s, m)
                    flush_sweep(s)
                units = [(s, m) for s in range(upfront, NSWEEP)
                         for m in range(16)]

                h_prev = statep.tile([128, 128], F16, tag="h0init")
                c_prev = statep.tile([128, 128], F32, tag="c0init")
                nc.gpsimd.memset(h_prev[:], 0.0)
                nc.gpsimd.memset(c_prev[:], 0.0)

                SIG = mybir.ActivationFunctionType.Sigmoid
                TANH = mybir.ActivationFunctionType.Tanh
                for t in range(T):
                    if t < len(units):
                        us, um = units[t]
                        if um == 0:
                            start_sweep(us)
                        emit_unit(us, um)
                        if um == 15:
                            flush_sweep(us)
                    xg_t = xgtp.tile([128, 512], F16)
                    nc.sync.dma_start(xg_t[:], xg_store[t])
                    # gate-group PSUM banks so f/i elementwise overlaps g/o MMs
                    ps_fi = rpsum.tile([128, 256], F32, tag="psfi")
                    ps_g = rpsum.tile([128, 128], F32, tag="psg")
                    ps_o = rpsum.tile([128, 128], F32, tag="pso")
                    outsl = ([(ps_fi, 32 * hb) for hb in range(4)]
                             + [(ps_fi, 128 + 32 * hb) for hb in range(4)]
                             + [(ps_g, 32 * hb) for hb in range(4)]
                             + [(ps_o, 32 * hb) for hb in range(4)])
                    for m in range(16):
                        pst, off = outsl[m]
                        for c in range(4):
                            nc.tensor.matmul(
                                pst[:, off:off + 32],
                                whh_t(c, m),
                                h_prev[:, 32 * c:32 * (c + 1)],
                                start=(c == 0), stop=(c == 3))
                    # f,i ready first: start the cell chain under g/o MMs
                    gfi = gatesp.tile([128, 256], F16, tag="gfi")
                    nc.vector.tensor_add(gfi[:], ps_fi[:], xg_t[:, 0:256])
                    sfi = gatesp.tile([128, 256], F16, tag="sfi")
                    nc.scalar.activation(sfi[:], gfi[:], SIG)
                    t1 = tmpp.tile([128, 128], F32, tag="t1")
                    nc.vector.tensor_mul(t1[:], sfi[:, 0:128], c_prev[:])
                    gg = tmpp.tile([128, 128], F16, tag="gg")
                    nc.vector.tensor_add(gg[:], ps_g[:], xg_t[:, 256:384])
                    tg = tmpp.tile([128, 128], F16, tag="tg")
                    nc.scalar.activation(tg[:], gg[:], TANH)
                    t2 = tmpp.tile([128, 128], F32, tag="t2")
                    nc.vector.tensor_mul(t2[:], sfi[:, 128:256], tg[:])
                    c_new = cellp.tile([128, 128], F32, tag="c")
                    nc.vector.tensor_add(c_new[:], t1[:], t2[:])
                    th = tmpp.tile([128, 128], F16, tag="th")
                    nc.scalar.activation(th[:], c_new[:], TANH)
                    go = tmpp.tile([128, 128], F16, tag="go")
                    nc.vector.tensor_add(go[:], ps_o[:], xg_t[:, 384:512])
                    so = tmpp.tile([128, 128], F16, tag="so")
                    nc.scalar.activation(so[:], go[:], SIG)
                    h_new = hbufp.tile([128, 128], F16, tag="h")
                    nc.vector.tensor_mul(h_new[:], so[:], th[:])
                    nc.sync.dma_start(hout_d[t], h_new[:])
                    h_prev, c_prev = h_new, c_new

    nc.compile()
    return nc


def _get_nc(nkc):
    if nkc not in _NC_CACHE:
        _NC_CACHE[nkc] = _build_launch(nkc)
    return _NC_CACHE[nkc]


def _prep_w(w, nkc):
    """[Din, 2048] -> [nkc, 16, 128, 128] f16 tiles, gate-col permuted."""
    wp = np.asarray(w, dtype=np.float32)[:, _PERM]
    return np.ascontiguousarray(
        wp.reshape(nkc, 128, 16, 128).transpose(0, 2, 1, 3)).astype(np.float16)


def _prep_biasq(b):
    """[2048] -> [4, 128, 2048] f32: biasq[q, p, (mm,tt,bb)] = b[perm[128*(4q+mm)+p]]."""
    bp = np.asarray(b, dtype=np.float32)[_PERM].reshape(16, 128)  # [m, p]
    out = np.empty((4, 128, 4, TSW, 32), dtype=np.float32)
    for q in range(4):
        for mm in range(4):
            out[q, :, mm, :, :] = bp[4 * q + mm][:, None, None]
    return out.reshape(4, 128, 2048)


def _prep_rhs_from_x(x_dir):
    """[B, T, IN] -> [16, 4, 128, 512] f16 with cols (t16, b32)."""
    xt = np.asarray(x_dir, dtype=np.float32).transpose(1, 2, 0)  # [T, IN, B]
    r = xt.reshape(NSWEEP, TSW, 4, 128, 32)
    return np.ascontiguousarray(r.transpose(0, 2, 3, 1, 4)).reshape(
        NSWEEP, 4, 128, 512).astype(np.float16)


def _prep_rhs_from_h0(h0):
    """[T, 128, 128] f16 (t, k, 32*hb+b) -> [16, 4, 128, 512]."""
    r = h0.reshape(NSWEEP, TSW, 128, 4, 32)
    return np.ascontiguousarray(r.transpose(0, 3, 2, 1, 4)).reshape(
        NSWEEP, 4, 128, 512)


def _unpack_h(h, reverse):
    """[T, 128, 128] f16 -> [B, T, H] f32."""
    a = h.astype(np.float32).reshape(T, 128, 4, 32).transpose(3, 0, 2, 1)
    a = np.ascontiguousarray(a).reshape(B, T, H)
    return a[:, ::-1, :] if reverse else a


def _run(nc, in_maps):
    res = bass_utils.run_bass_kernel_spmd(
        nc, in_maps, core_ids=list(range(NCORES)), trace=False)
    return res


def kernel(x, w_ih0f, w_hh0f, b0f, w_ih0b, w_hh0b, b0b,
           w_ih1f, w_hh1f, b1f, w_ih1b, w_hh1b, b1b):
    x = np.asarray(x, dtype=np.float32)
    xr = x[:, ::-1, :]

    # ---- launch 1: layer 0 ----
    nc1 = _get_nc(4)
    fwd_in = {
        "rhs": _prep_rhs_from_x(x),
        "wih": _prep_w(w_ih0f, 4),
        "whh": _prep_w(w_hh0f, 4),
        "biasq": _prep_biasq(b0f),
    }
    bwd_in = {
        "rhs": _prep_rhs_from_x(xr),
        "wih": _prep_w(w_ih0b, 4),
        "whh": _prep_w(w_hh0b, 4),
        "biasq": _prep_biasq(b0b),
    }
    in_maps = [fwd_in if c % 2 == 0 else bwd_in for c in range(NCORES)]
    res1 = _run(nc1, in_maps)
    h0f = res1.results[0]["hout"]  # [T,128,128] f16, canonical time
    h0b = res1.results[1]["hout"]  # bwd-scan order (reversed time)

    # ---- launch 2: layer 1 ----
    nc2 = _get_nc(8)
    # fwd consumes [h0f(own), h0b reversed-to-canonical]; bwd the mirror.
    rhs_f = np.concatenate(
        [_prep_rhs_from_h0(h0f), _prep_rhs_from_h0(h0b[::-1])], axis=1)
    rhs_b = np.concatenate(
        [_prep_rhs_from_h0(h0b), _prep_rhs_from_h0(h0f[::-1])], axis=1)
    wih1f_t = _prep_w(w_ih1f, 8)           # κ0-3 = rows 0:512 (h0f half)
    wih1b_t = _prep_w(w_ih1b, 8)
    wih1b_t = np.concatenate([wih1b_t[4:], wih1b_t[:4]], axis=0)  # own half first
    fwd_in2 = {"rhs": rhs_f, "wih": wih1f_t, "whh": _prep_w(w_hh1f, 4),
               "biasq": _prep_biasq(b1f)}
    bwd_in2 = {"rhs": rhs_b, "wih": wih1b_t, "whh": _prep_w(w_hh1b, 4),
               "biasq": _prep_biasq(b1b)}
    in_maps2 = [fwd_in2 if c % 2 == 0 else bwd_in2 for c in range(NCORES)]
    res2 = _run(nc2, in_maps2)
    h1f = res2.results[0]["hout"]
    h1b = res2.results[1]["hout"]

    out = np.concatenate(
        [_unpack_h(h1f, False), _unpack_h(h1b, True)], axis=2)
    return np.ascontiguousarray(out).astype(np.float32)



# revision 3
# speedup vs baseline: 1.1424x; 1.1424x over previous
"""Bidirectional 2-layer LSTM (B=32, T=256, IN=H=512) on 8 Trainium2 cores.

Strategy:
  - Directions are anti-aligned in time, so fwd/bwd run on separate cores
    (even cores = fwd, odd = bwd) with bwd fed time-reversed inputs.
  - Two SPMD launches: launch 1 computes layer-0 input gates + recurrence,
    launch 2 (after a host roundtrip that exchanges/reverses h0 between
    directions) computes layer-1 the same way. Both launches share one NEFF
    builder; the only structural difference is the contraction depth of the
    input-gate GEMM (512 vs 1024).
  - Per-core layout keeps the hidden state as [Hw=128 part, (hb,b)=128 free]
    fp16 so the recurrent matmul (w_hh stationary, h moving) needs no
    transposes anywhere in the 256-step serial chain.
  - Gate tile order is [f, i, o, g] so one Sigmoid covers f,i,o and one Tanh
    covers g. Cell state stays fp32; matmul operands fp16 with fp32 PSUM.
"""

import os
import sys

for _p in ("/opt/trn_rl_repo", "/root/.axon_site/_ro/trn_rl_repo"):
    if os.path.isdir(_p) and _p not in sys.path:
        sys.path.insert(0, _p)

import numpy as np

import concourse.bass as bass
import concourse.bacc as bacc
import concourse.tile as tile
import concourse.mybir as mybir
import concourse.bass_utils as bass_utils

NCORES = 8
B, T, IN, H = 32, 256, 512, 512
T = int(os.environ.get("LSTM_T", T))  # test override; harness uses 256
NSWEEP = T // 16     # t-sweeps in the input-gate GEMM
TSW = 16             # timesteps per sweep
F16 = mybir.dt.float16
F32 = mybir.dt.float32

# gate tile order [f, i, g, o]; OG maps tile-group -> original gate index
# in the reference's [i, f, g, o] column order.
OG = (1, 0, 2, 3)
_PERM = np.concatenate(
    [512 * OG[m // 4] + 128 * (m % 4) + np.arange(128) for m in range(16)]
)

_NC_CACHE = {}


def _build_launch(nkc):
    """One direction-layer: xg GEMM (nkc K-chunks of 128) + 256-step LSTM
    recurrence. Inputs (per core):
      rhs   [NSWEEP, nkc, 128, 512] f16 -- moving operand tiles, (t16,b32) cols
      wih   [nkc, 16, 128, 128] f16     -- input-weight tiles (col-permuted)
      whh   [4, 16, 128, 128] f16       -- recurrent-weight tiles
      biasq [4, 128, 2048] f32          -- bias pre-broadcast per m-quarter
    Output:
      hout  [T, 128, 128] f16           -- h_t in [Hw, (hb,b)] layout
    """
    nc = bacc.Bacc("TRN2", target_bir_lowering=False, debug=False,
                   enable_asserts=True, num_devices=NCORES)
    rhs_d = nc.dram_tensor("rhs", [NSWEEP, nkc, 128, 512], F16,
                           kind="ExternalInput")
    wih_d = nc.dram_tensor("wih", [nkc, 16, 128, 128], F16,
                           kind="ExternalInput")
    whh_d = nc.dram_tensor("whh", [4, 16, 128, 128], F16,
                           kind="ExternalInput")
    biasq_d = nc.dram_tensor("biasq", [4, 128, 2048], F32,
                             kind="ExternalInput")
    hout_d = nc.dram_tensor("hout", [T, 128, 128], F16,
                            kind="ExternalOutput")

    with tile.TileContext(nc) as tc:
        with (
            tc.tile_pool(name="wpool", bufs=1) as wpool,
            tc.tile_pool(name="dram", bufs=1, space="DRAM") as dram,
        ):
            # resident weights / bias
            wih_sb = wpool.tile([128, nkc * 16 * 128], F16)
            whh_sb = wpool.tile([128, 4 * 16 * 128], F16)
            biasq_sb = wpool.tile([128, 4 * 2048], F32)
            nc.sync.dma_start(
                wih_sb[:].rearrange("k (c m j) -> k c m j",
                                      c=nkc, m=16, j=128),
                wih_d.ap().rearrange("c m k j -> k c m j"))
            nc.sync.dma_start(
                whh_sb[:].rearrange("k (c m j) -> k c m j",
                                      c=4, m=16, j=128),
                whh_d.ap().rearrange("c m k j -> k c m j"))
            nc.sync.dma_start(
                biasq_sb[:].rearrange("k (q j) -> k q j", q=4, j=2048),
                biasq_d.ap().rearrange("q k j -> k q j"))

            def wih_t(c, m):
                o = (c * 16 + m) * 128
                return wih_sb[:, o:o + 128]

            def whh_t(c, m):
                o = (c * 16 + m) * 128
                return whh_sb[:, o:o + 128]

            xg_store = dram.tile([T, 128, 512], F16)

            # ---- xg GEMM interleaved with the recurrence ----
            # The rec step leaves the PE idle during its elementwise tail;
            # emitting one m-tile of a future sweep's GEMM per rec step fills
            # those gaps, making the input-gate GEMM (nearly) free. Two
            # sweeps are emitted up front so xg stays 2 sweeps ahead.
            with (
                tc.tile_pool(name="rt", bufs=2 * nkc) as rtp,
                tc.tile_pool(name="apsum", bufs=2, space="PSUM") as apsum,
                tc.tile_pool(name="xgsb", bufs=2) as xgp,
                tc.tile_pool(name="state", bufs=1) as statep,
                tc.tile_pool(name="xgt", bufs=4) as xgtp,
                tc.tile_pool(name="rpsum", bufs=2, space="PSUM") as rpsum,
                tc.tile_pool(name="gates", bufs=2) as gatesp,
                tc.tile_pool(name="cell", bufs=2) as cellp,
                tc.tile_pool(name="hbuf", bufs=2) as hbufp,
                tc.tile_pool(name="tmp", bufs=4) as tmpp,
            ):
                sweep_ctx = {}

                def start_sweep(s):
                    rts = []
                    for c in range(nkc):
                        rt = rtp.tile([128, 512], F16, name="rt")
                        nc.sync.dma_start(rt[:], rhs_d[s, c])
                        rts.append(rt)
                    xg_sb = xgp.tile([128, TSW * 512], F16, name="xg_sb")
                    sweep_ctx[s] = (rts, xg_sb)

                def emit_unit(s, m):
                    rts, xg_sb = sweep_ctx[s]
                    ps = apsum.tile([128, 512], F32, name="aps", tag="aps")
                    for c in range(nkc):
                        nc.tensor.matmul(ps[:], wih_t(c, m), rts[c][:],
                                         start=(c == 0), stop=(c == nkc - 1))
                    src = ps[:].rearrange("k (t b) -> k t b", t=TSW, b=32)
                    dst = xg_sb[:].rearrange("k (t m b) -> k m t b",
                                             t=TSW, m=16, b=32)[:, m]
                    q, mm = m // 4, m % 4
                    bias = biasq_sb[:, 2048 * q:2048 * (q + 1)].rearrange(
                        "k (m t b) -> k m t b", m=4, t=TSW, b=32)[:, mm]
                    nc.vector.tensor_add(dst, src, bias)

                def flush_sweep(s):
                    _, xg_sb = sweep_ctx.pop(s)
                    nc.sync.dma_start(
                        xg_store[TSW * s:TSW * (s + 1)].rearrange(
                            "t k c -> k t c"),
                        xg_sb[:].rearrange("k (t c) -> k t c",
                                           t=TSW, c=512))

                upfront = min(2, NSWEEP)
                for s in range(upfront):
                    start_sweep(s)
                    for m in range(16):
                        emit_unit(s, m)
                    flush_sweep(s)
                units = [(s, m) for s in range(upfront, NSWEEP)
                         for m in range(16)]

                h_prev = statep.tile([128, 128], F16, tag="h0init")
                c_prev = statep.tile([128, 128], F16, tag="c0init")
                nc.gpsimd.memset(h_prev[:], 0.0)
                nc.gpsimd.memset(c_prev[:], 0.0)

                SIG = mybir.ActivationFunctionType.Sigmoid
                TANH = mybir.ActivationFunctionType.Tanh
                for t in range(T):
                    if t < len(units):
                        us, um = units[t]
                        if um == 0:
                            start_sweep(us)
                        emit_unit(us, um)
                        if um == 15:
                            flush_sweep(us)
                    xg_t = xgtp.tile([128, 512], F16)
                    nc.sync.dma_start(xg_t[:], xg_store[t])
                    # gate-group PSUM banks so f/i elementwise overlaps g/o MMs
                    ps_fi = rpsum.tile([128, 256], F32, tag="psfi")
                    ps_g = rpsum.tile([128, 128], F32, tag="psg")
                    ps_o = rpsum.tile([128, 128], F32, tag="pso")
                    outsl = ([(ps_fi, 32 * hb) for hb in range(4)]
                             + [(ps_fi, 128 + 32 * hb) for hb in range(4)]
                             + [(ps_g, 32 * hb) for hb in range(4)]
                             + [(ps_o, 32 * hb) for hb in range(4)])
                    for m in range(16):
                        pst, off = outsl[m]
                        for c in range(4):
                            nc.tensor.matmul(
                                pst[:, off:off + 32],
                                whh_t(c, m),
                                h_prev[:, 32 * c:32 * (c + 1)],
                                start=(c == 0), stop=(c == 3))
                    # f,i ready first: start the cell chain under g/o MMs.
                    # Cell chain (t1/t2/c_new) runs on GpSimd so the DVE queue
                    # only carries the PSUM-source adds + final h mul; c stays
                    # f16 (error stays ~1e-3, well under the 2e-2 gate).
                    gfi = gatesp.tile([128, 256], F16, tag="gfi")
                    nc.vector.tensor_add(gfi[:], ps_fi[:], xg_t[:, 0:256])
                    sfi = gatesp.tile([128, 256], F16, tag="sfi")
                    nc.scalar.activation(sfi[:], gfi[:], SIG)
                    t1 = tmpp.tile([128, 128], F16, tag="t1")
                    nc.gpsimd.tensor_mul(t1[:], sfi[:, 0:128], c_prev[:])
                    gg = tmpp.tile([128, 128], F16, tag="gg")
                    nc.vector.tensor_add(gg[:], ps_g[:], xg_t[:, 256:384])
                    tg = tmpp.tile([128, 128], F16, tag="tg")
                    nc.scalar.activation(tg[:], gg[:], TANH)
                    t2 = tmpp.tile([128, 128], F16, tag="t2")
                    nc.gpsimd.tensor_mul(t2[:], sfi[:, 128:256], tg[:])
                    c_new = cellp.tile([128, 128], F16, tag="c")
                    nc.gpsimd.tensor_add(c_new[:], t1[:], t2[:])
                    th = tmpp.tile([128, 128], F16, tag="th")
                    nc.scalar.activation(th[:], c_new[:], TANH)
                    go = tmpp.tile([128, 128], F16, tag="go")
                    nc.vector.tensor_add(go[:], ps_o[:], xg_t[:, 384:512])
                    so = tmpp.tile([128, 128], F16, tag="so")
                    nc.scalar.activation(so[:], go[:], SIG)
                    h_new = hbufp.tile([128, 128], F16, tag="h")
                    nc.vector.tensor_mul(h_new[:], so[:], th[:])
                    nc.sync.dma_start(hout_d[t], h_new[:])
                    h_prev, c_prev = h_new, c_new

    nc.compile()
    return nc


def _get_nc(nkc):
    if nkc not in _NC_CACHE:
        _NC_CACHE[nkc] = _build_launch(nkc)
    return _NC_CACHE[nkc]


def _prep_w(w, nkc):
    """[Din, 2048] -> [nkc, 16, 128, 128] f16 tiles, gate-col permuted."""
    wp = np.asarray(w, dtype=np.float32)[:, _PERM]
    return np.ascontiguousarray(
        wp.reshape(nkc, 128, 16, 128).transpose(0, 2, 1, 3)).astype(np.float16)


def _prep_biasq(b):
    """[2048] -> [4, 128, 2048] f32: biasq[q, p, (mm,tt,bb)] = b[perm[128*(4q+mm)+p]]."""
    bp = np.asarray(b, dtype=np.float32)[_PERM].reshape(16, 128)  # [m, p]
    out = np.empty((4, 128, 4, TSW, 32), dtype=np.float32)
    for q in range(4):
        for mm in range(4):
            out[q, :, mm, :, :] = bp[4 * q + mm][:, None, None]
    return out.reshape(4, 128, 2048)


def _prep_rhs_from_x(x_dir):
    """[B, T, IN] -> [16, 4, 128, 512] f16 with cols (t16, b32)."""
    xt = np.asarray(x_dir, dtype=np.float32).transpose(1, 2, 0)  # [T, IN, B]
    r = xt.reshape(NSWEEP, TSW, 4, 128, 32)
    return np.ascontiguousarray(r.transpose(0, 2, 3, 1, 4)).reshape(
        NSWEEP, 4, 128, 512).astype(np.float16)


def _prep_rhs_from_h0(h0):
    """[T, 128, 128] f16 (t, k, 32*hb+b) -> [16, 4, 128, 512]."""
    r = h0.reshape(NSWEEP, TSW, 128, 4, 32)
    return np.ascontiguousarray(r.transpose(0, 3, 2, 1, 4)).reshape(
        NSWEEP, 4, 128, 512)


def _unpack_h(h, reverse):
    """[T, 128, 128] f16 -> [B, T, H] f32."""
    a = h.astype(np.float32).reshape(T, 128, 4, 32).transpose(3, 0, 2, 1)
    a = np.ascontiguousarray(a).reshape(B, T, H)
    return a[:, ::-1, :] if reverse else a


def _run(nc, in_maps):
    res = bass_utils.run_bass_kernel_spmd(
        nc, in_maps, core_ids=list(range(NCORES)), trace=False)
    return res


def kernel(x, w_ih0f, w_hh0f, b0f, w_ih0b, w_hh0b, b0b,
           w_ih1f, w_hh1f, b1f, w_ih1b, w_hh1b, b1b):
    x = np.asarray(x, dtype=np.float32)
    xr = x[:, ::-1, :]

    # ---- launch 1: layer 0 ----
    nc1 = _get_nc(4)
    fwd_in = {
        "rhs": _prep_rhs_from_x(x),
        "wih": _prep_w(w_ih0f, 4),
        "whh": _prep_w(w_hh0f, 4),
        "biasq": _prep_biasq(b0f),
    }
    bwd_in = {
        "rhs": _prep_rhs_from_x(xr),
        "wih": _prep_w(w_ih0b, 4),
        "whh": _prep_w(w_hh0b, 4),
        "biasq": _prep_biasq(b0b),
    }
    in_maps = [fwd_in if c % 2 == 0 else bwd_in for c in range(NCORES)]
    res1 = _run(nc1, in_maps)
    h0f = res1.results[0]["hout"]  # [T,128,128] f16, canonical time
    h0b = res1.results[1]["hout"]  # bwd-scan order (reversed time)

    # ---- launch 2: layer 1 ----
    nc2 = _get_nc(8)
    # fwd consumes [h0f(own), h0b reversed-to-canonical]; bwd the mirror.
    rhs_f = np.concatenate(
        [_prep_rhs_from_h0(h0f), _prep_rhs_from_h0(h0b[::-1])], axis=1)
    rhs_b = np.concatenate(
        [_prep_rhs_from_h0(h0b), _prep_rhs_from_h0(h0f[::-1])], axis=1)
    wih1f_t = _prep_w(w_ih1f, 8)           # κ0-3 = rows 0:512 (h0f half)
    wih1b_t = _prep_w(w_ih1b, 8)
    wih1b_t = np.concatenate([wih1b_t[4:], wih1b_t[:4]], axis=0)  # own half first
    fwd_in2 = {"rhs": rhs_f, "wih": wih1f_t, "whh": _prep_w(w_hh1f, 4),
               "biasq": _prep_biasq(b1f)}
    bwd_in2 = {"rhs": rhs_b, "wih": wih1b_t, "whh": _prep_w(w_hh1b, 4),
               "biasq": _prep_biasq(b1b)}
    in_maps2 = [fwd_in2 if c % 2 == 0 else bwd_in2 for c in range(NCORES)]
    res2 = _run(nc2, in_maps2)
    h1f = res2.results[0]["hout"]
    h1b = res2.results[1]["hout"]

    out = np.concatenate(
        [_unpack_h(h1f, False), _unpack_h(h1b, True)], axis=2)
    return np.ascontiguousarray(out).astype(np.float32)



# revision 5
# speedup vs baseline: 1.1437x; 1.0012x over previous
"""Bidirectional 2-layer LSTM (B=32, T=256, IN=H=512) on 8 Trainium2 cores.

Strategy:
  - Directions are anti-aligned in time, so fwd/bwd run on separate cores
    (even cores = fwd, odd = bwd) with bwd fed time-reversed inputs.
  - Two SPMD launches: launch 1 computes layer-0 input gates + recurrence,
    launch 2 (after a host roundtrip that exchanges/reverses h0 between
    directions) computes layer-1 the same way. Both launches share one NEFF
    builder; the only structural difference is the contraction depth of the
    input-gate GEMM (512 vs 1024).
  - Per-core layout keeps the hidden state as [Hw=128 part, (hb,b)=128 free]
    fp16 so the recurrent matmul (w_hh stationary, h moving) needs no
    transposes anywhere in the 256-step serial chain.
  - Gate tile order is [f, i, o, g] so one Sigmoid covers f,i,o and one Tanh
    covers g. Cell state is fp16 (DVE 2x mode; rel err stays ~1.4e-3);
    matmul operands fp16 with fp32 PSUM.
"""

import os
import sys

for _p in ("/opt/trn_rl_repo", "/root/.axon_site/_ro/trn_rl_repo"):
    if os.path.isdir(_p) and _p not in sys.path:
        sys.path.insert(0, _p)

import numpy as np

import concourse.bass as bass
import concourse.bacc as bacc
import concourse.tile as tile
import concourse.mybir as mybir
import concourse.bass_utils as bass_utils

NCORES = 8
B, T, IN, H = 32, 256, 512, 512
T = int(os.environ.get("LSTM_T", T))  # test override; harness uses 256
NSWEEP = T // 16     # t-sweeps in the input-gate GEMM
TSW = 16             # timesteps per sweep
F16 = mybir.dt.float16
F32 = mybir.dt.float32

# gate tile order [f, i, g, o]; OG maps tile-group -> original gate index
# in the reference's [i, f, g, o] column order.
OG = (1, 0, 2, 3)
_PERM = np.concatenate(
    [512 * OG[m // 4] + 128 * (m % 4) + np.arange(128) for m in range(16)]
)

_NC_CACHE = {}


def _build_launch(nkc):
    """One direction-layer: xg GEMM (nkc K-chunks of 128) + 256-step LSTM
    recurrence. Inputs (per core):
      rhs   [NSWEEP, nkc, 128, 512] f16 -- moving operand tiles, (t16,b32) cols
      wih   [nkc, 16, 128, 128] f16     -- input-weight tiles (col-permuted)
      whh   [4, 16, 128, 128] f16       -- recurrent-weight tiles
      biasq [4, 128, 2048] f32          -- bias pre-broadcast per m-quarter
    Output:
      hout  [T, 128, 128] f16           -- h_t in [Hw, (hb,b)] layout
    """
    nc = bacc.Bacc("TRN2", target_bir_lowering=False, debug=False,
                   enable_asserts=True, num_devices=NCORES)
    rhs_d = nc.dram_tensor("rhs", [NSWEEP, nkc, 128, 512], F16,
                           kind="ExternalInput")
    wih_d = nc.dram_tensor("wih", [nkc, 16, 128, 128], F16,
                           kind="ExternalInput")
    whh_d = nc.dram_tensor("whh", [4, 16, 128, 128], F16,
                           kind="ExternalInput")
    biasq_d = nc.dram_tensor("biasq", [4, 128, 2048], F32,
                             kind="ExternalInput")
    hout_d = nc.dram_tensor("hout", [T, 128, 128], F16,
                            kind="ExternalOutput")

    with tile.TileContext(nc) as tc:
        with (
            tc.tile_pool(name="wpool", bufs=1) as wpool,
            tc.tile_pool(name="dram", bufs=1, space="DRAM") as dram,
        ):
            # resident weights / bias
            wih_sb = wpool.tile([128, nkc * 16 * 128], F16)
            whh_sb = wpool.tile([128, 4 * 16 * 128], F16)
            biasq_sb = wpool.tile([128, 4 * 2048], F32)
            nc.sync.dma_start(
                wih_sb[:].rearrange("k (c m j) -> k c m j",
                                      c=nkc, m=16, j=128),
                wih_d.ap().rearrange("c m k j -> k c m j"))
            nc.sync.dma_start(
                whh_sb[:].rearrange("k (c m j) -> k c m j",
                                      c=4, m=16, j=128),
                whh_d.ap().rearrange("c m k j -> k c m j"))
            nc.sync.dma_start(
                biasq_sb[:].rearrange("k (q j) -> k q j", q=4, j=2048),
                biasq_d.ap().rearrange("q k j -> k q j"))

            def wih_t(c, m):
                o = (c * 16 + m) * 128
                return wih_sb[:, o:o + 128]

            def whh_t(c, m):
                o = (c * 16 + m) * 128
                return whh_sb[:, o:o + 128]

            xg_store = dram.tile([T, 128, 512], F16)

            # ---- xg GEMM interleaved with the recurrence ----
            # The rec step leaves the PE idle during its elementwise tail;
            # emitting one m-tile of a future sweep's GEMM per rec step fills
            # those gaps, making the input-gate GEMM (nearly) free. Two
            # sweeps are emitted up front so xg stays 2 sweeps ahead.
            with (
                tc.tile_pool(name="rt", bufs=2 * nkc) as rtp,
                tc.tile_pool(name="apsum", bufs=2, space="PSUM") as apsum,
                tc.tile_pool(name="xgsb", bufs=2) as xgp,
                tc.tile_pool(name="state", bufs=1) as statep,
                tc.tile_pool(name="xgt", bufs=4) as xgtp,
                tc.tile_pool(name="rpsum", bufs=2, space="PSUM") as rpsum,
                tc.tile_pool(name="gates", bufs=2) as gatesp,
                tc.tile_pool(name="cell", bufs=2) as cellp,
                tc.tile_pool(name="hbuf", bufs=2) as hbufp,
                tc.tile_pool(name="tmp", bufs=4) as tmpp,
            ):
                sweep_ctx = {}

                def start_sweep(s):
                    rts = []
                    for c in range(nkc):
                        rt = rtp.tile([128, 512], F16, name="rt")
                        nc.sync.dma_start(rt[:], rhs_d[s, c])
                        rts.append(rt)
                    xg_sb = xgp.tile([128, TSW * 512], F16, name="xg_sb")
                    sweep_ctx[s] = (rts, xg_sb)

                def emit_unit(s, m):
                    rts, xg_sb = sweep_ctx[s]
                    ps = apsum.tile([128, 512], F32, name="aps", tag="aps")
                    for c in range(nkc):
                        nc.tensor.matmul(ps[:], wih_t(c, m), rts[c][:],
                                         start=(c == 0), stop=(c == nkc - 1))
                    src = ps[:].rearrange("k (t b) -> k t b", t=TSW, b=32)
                    dst = xg_sb[:].rearrange("k (t m b) -> k m t b",
                                             t=TSW, m=16, b=32)[:, m]
                    q, mm = m // 4, m % 4
                    bias = biasq_sb[:, 2048 * q:2048 * (q + 1)].rearrange(
                        "k (m t b) -> k m t b", m=4, t=TSW, b=32)[:, mm]
                    nc.vector.tensor_add(dst, src, bias)

                def flush_sweep(s):
                    _, xg_sb = sweep_ctx.pop(s)
                    nc.sync.dma_start(
                        xg_store[TSW * s:TSW * (s + 1)].rearrange(
                            "t k c -> k t c"),
                        xg_sb[:].rearrange("k (t c) -> k t c",
                                           t=TSW, c=512))

                upfront = min(2, NSWEEP)
                for s in range(upfront):
                    start_sweep(s)
                    for m in range(16):
                        emit_unit(s, m)
                    flush_sweep(s)
                units = [(s, m) for s in range(upfront, NSWEEP)
                         for m in range(16)]

                h_prev = statep.tile([128, 128], F16, tag="h0init")
                c_prev = statep.tile([128, 128], F16, tag="c0init")
                nc.gpsimd.memset(h_prev[:], 0.0)
                nc.gpsimd.memset(c_prev[:], 0.0)

                SIG = mybir.ActivationFunctionType.Sigmoid
                TANH = mybir.ActivationFunctionType.Tanh
                for t in range(T):
                    if t < len(units):
                        us, um = units[t]
                        if um == 0:
                            start_sweep(us)
                        emit_unit(us, um)
                        if um == 15:
                            flush_sweep(us)
                    xg_t = xgtp.tile([128, 512], F16)
                    nc.sync.dma_start(xg_t[:], xg_store[t])
                    # gate-group PSUM banks so f/i elementwise overlaps g/o MMs
                    ps_fi = rpsum.tile([128, 256], F32, tag="psfi")
                    ps_g = rpsum.tile([128, 128], F32, tag="psg")
                    ps_o = rpsum.tile([128, 128], F32, tag="pso")
                    outsl = ([(ps_fi, 32 * hb) for hb in range(4)]
                             + [(ps_fi, 128 + 32 * hb) for hb in range(4)]
                             + [(ps_g, 32 * hb) for hb in range(4)]
                             + [(ps_o, 32 * hb) for hb in range(4)])
                    for m in range(16):
                        pst, off = outsl[m]
                        for c in range(4):
                            nc.tensor.matmul(
                                pst[:, off:off + 32],
                                whh_t(c, m),
                                h_prev[:, 32 * c:32 * (c + 1)],
                                start=(c == 0), stop=(c == 3))
                    # f,i ready first: start the cell chain under g/o MMs.
                    # c stays f16: halves DVE cost of the cell chain (2x mode)
                    # and the f16 rounding keeps rel err ~1.4e-3, well under
                    # the 2e-2 gate.
                    gfi = gatesp.tile([128, 256], F16, tag="gfi")
                    nc.vector.tensor_add(gfi[:], ps_fi[:], xg_t[:, 0:256])
                    sfi = gatesp.tile([128, 256], F16, tag="sfi")
                    nc.scalar.activation(sfi[:], gfi[:], SIG)
                    t1 = tmpp.tile([128, 128], F16, tag="t1")
                    nc.vector.tensor_mul(t1[:], sfi[:, 0:128], c_prev[:])
                    gg = tmpp.tile([128, 128], F16, tag="gg")
                    nc.vector.tensor_add(gg[:], ps_g[:], xg_t[:, 256:384])
                    tg = tmpp.tile([128, 128], F16, tag="tg")
                    nc.scalar.activation(tg[:], gg[:], TANH)
                    t2 = tmpp.tile([128, 128], F16, tag="t2")
                    nc.vector.tensor_mul(t2[:], sfi[:, 128:256], tg[:])
                    c_new = cellp.tile([128, 128], F16, tag="c")
                    nc.vector.tensor_add(c_new[:], t1[:], t2[:])
                    th = tmpp.tile([128, 128], F16, tag="th")
                    nc.scalar.activation(th[:], c_new[:], TANH)
                    go = tmpp.tile([128, 128], F16, tag="go")
                    nc.vector.tensor_add(go[:], ps_o[:], xg_t[:, 384:512])
                    so = tmpp.tile([128, 128], F16, tag="so")
                    nc.scalar.activation(so[:], go[:], SIG)
                    h_new = hbufp.tile([128, 128], F16, tag="h")
                    nc.vector.tensor_mul(h_new[:], so[:], th[:])
                    nc.sync.dma_start(hout_d[t], h_new[:])
                    h_prev, c_prev = h_new, c_new

    nc.compile()
    return nc


def _get_nc(nkc):
    if nkc not in _NC_CACHE:
        _NC_CACHE[nkc] = _build_launch(nkc)
    return _NC_CACHE[nkc]


def _prep_w(w, nkc):
    """[Din, 2048] -> [nkc, 16, 128, 128] f16 tiles, gate-col permuted."""
    wp = np.asarray(w, dtype=np.float32)[:, _PERM]
    return np.ascontiguousarray(
        wp.reshape(nkc, 128, 16, 128).transpose(0, 2, 1, 3)).astype(np.float16)


def _prep_biasq(b):
    """[2048] -> [4, 128, 2048] f32: biasq[q, p, (mm,tt,bb)] = b[perm[128*(4q+mm)+p]]."""
    bp = np.asarray(b, dtype=np.float32)[_PERM].reshape(16, 128)  # [m, p]
    out = np.empty((4, 128, 4, TSW, 32), dtype=np.float32)
    for q in range(4):
        for mm in range(4):
            out[q, :, mm, :, :] = bp[4 * q + mm][:, None, None]
    return out.reshape(4, 128, 2048)


def _prep_rhs_from_x(x_dir):
    """[B, T, IN] -> [16, 4, 128, 512] f16 with cols (t16, b32)."""
    xt = np.asarray(x_dir, dtype=np.float32).transpose(1, 2, 0)  # [T, IN, B]
    r = xt.reshape(NSWEEP, TSW, 4, 128, 32)
    return np.ascontiguousarray(r.transpose(0, 2, 3, 1, 4)).reshape(
        NSWEEP, 4, 128, 512).astype(np.float16)


def _prep_rhs_from_h0(h0):
    """[T, 128, 128] f16 (t, k, 32*hb+b) -> [16, 4, 128, 512]."""
    r = h0.reshape(NSWEEP, TSW, 128, 4, 32)
    return np.ascontiguousarray(r.transpose(0, 3, 2, 1, 4)).reshape(
        NSWEEP, 4, 128, 512)


def _unpack_h(h, reverse):
    """[T, 128, 128] f16 -> [B, T, H] f32."""
    a = h.astype(np.float32).reshape(T, 128, 4, 32).transpose(3, 0, 2, 1)
    a = np.ascontiguousarray(a).reshape(B, T, H)
    return a[:, ::-1, :] if reverse else a


def _run(nc, in_maps):
    res = bass_utils.run_bass_kernel_spmd(
        nc, in_maps, core_ids=list(range(NCORES)), trace=False)
    return res


def kernel(x, w_ih0f, w_hh0f, b0f, w_ih0b, w_hh0b, b0b,
           w_ih1f, w_hh1f, b1f, w_ih1b, w_hh1b, b1b):
    x = np.asarray(x, dtype=np.float32)
    xr = x[:, ::-1, :]

    # ---- launch 1: layer 0 ----
    nc1 = _get_nc(4)
    fwd_in = {
        "rhs": _prep_rhs_from_x(x),
        "wih": _prep_w(w_ih0f, 4),
        "whh": _prep_w(w_hh0f, 4),
        "biasq": _prep_biasq(b0f),
    }
    bwd_in = {
        "rhs": _prep_rhs_from_x(xr),
        "wih": _prep_w(w_ih0b, 4),
        "whh": _prep_w(w_hh0b, 4),
        "biasq": _prep_biasq(b0b),
    }
    in_maps = [fwd_in if c % 2 == 0 else bwd_in for c in range(NCORES)]
    res1 = _run(nc1, in_maps)
    h0f = res1.results[0]["hout"]  # [T,128,128] f16, canonical time
    h0b = res1.results[1]["hout"]  # bwd-scan order (reversed time)

    # ---- launch 2: layer 1 ----
    nc2 = _get_nc(8)
    # fwd consumes [h0f(own), h0b reversed-to-canonical]; bwd the mirror.
    rhs_f = np.concatenate(
        [_prep_rhs_from_h0(h0f), _prep_rhs_from_h0(h0b[::-1])], axis=1)
    rhs_b = np.concatenate(
        [_prep_rhs_from_h0(h0b), _prep_rhs_from_h0(h0f[::-1])], axis=1)
    wih1f_t = _prep_w(w_ih1f, 8)           # κ0-3 = rows 0:512 (h0f half)
    wih1b_t = _prep_w(w_ih1b, 8)
    wih1b_t = np.concatenate([wih1b_t[4:], wih1b_t[:4]], axis=0)  # own half first
    fwd_in2 = {"rhs": rhs_f, "wih": wih1f_t, "whh": _prep_w(w_hh1f, 4),
               "biasq": _prep_biasq(b1f)}
    bwd_in2 = {"rhs": rhs_b, "wih": wih1b_t, "whh": _prep_w(w_hh1b, 4),
               "biasq": _prep_biasq(b1b)}
    in_maps2 = [fwd_in2 if c % 2 == 0 else bwd_in2 for c in range(NCORES)]
    res2 = _run(nc2, in_maps2)
    h1f = res2.results[0]["hout"]
    h1b = res2.results[1]["hout"]

    out = np.concatenate(
        [_unpack_h(h1f, False), _unpack_h(h1b, True)], axis=2)
    return np.ascontiguousarray(out).astype(np.float32)

